# revision 5
# baseline (speedup 1.0000x reference)
"""Trainium2 Bass kernel for the Aligner module.

Computes, per batch b:
    g = sigmoid(conv2(relu(conv1(relu(x)))))          # [T] monotone gate
    ends = cumsum(g * valid_mask)                     # [T]
    centers = ends - 0.5*g ; aligned_len = ends[-1]
    w = softmax_t(-(centers[t]-pos[l])^2 / 10) masked # [L, T]
    out = w @ x^T                                     # [L, C]

Data-parallel over 8 NeuronCores: 4 batch elements per core, weights
replicated.  All matmuls run in bf16 (validated: output rel err ~2.7e-3
vs fp32 reference).  Host pre-transposes x to [T, C] so the final einsum
needs no on-device transpose of x; softmax weights are transposed on
device via DMA-transpose (bf16).
"""

import os
import sys

import numpy as np

B, C, T = 32, 512, 2048
L = 80
SIGMA2 = 10.0
NCORES = 8
BPC = B // NCORES  # batch elements per core

_cache = {}
LAST_RESULTS = None  # BassKernelResults of the most recent run (for profiling)


def _concourse():
    if "mods" in _cache:
        return _cache["mods"]
    if "/opt/trn_rl_repo" not in sys.path:
        sys.path.insert(0, "/opt/trn_rl_repo")
    import concourse.bass as bass
    import concourse.bacc as bacc
    import concourse.tile as tile
    from concourse import mybir
    from concourse import bass_utils

    _cache["mods"] = (bass, bacc, tile, mybir, bass_utils)
    return _cache["mods"]


def _build():
    """Build + compile the per-core Bass graph (cached)."""
    if "nc" in _cache:
        return _cache["nc"]
    bass, bacc, tile, mybir, _ = _concourse()
    from contextlib import ExitStack

    dt = mybir.dt
    f32, bf16 = dt.float32, dt.bfloat16
    Alu = mybir.AluOpType
    Act = mybir.ActivationFunctionType

    nc = bacc.Bacc("TRN2", target_bir_lowering=False)

    xbf = nc.declare_dram_parameter("xbf", [BPC, C, T], bf16, isOutput=False)
    xt = nc.declare_dram_parameter("xt", [BPC, T, C], bf16, isOutput=False)
    w1t = nc.declare_dram_parameter("w1t", [C, C], bf16, isOutput=False)  # [c, o]
    w2 = nc.declare_dram_parameter("w2", [128, 4], bf16, isOutput=False)  # col oc
    b1 = nc.declare_dram_parameter("b1", [128, 4], f32, isOutput=False)  # col oc
    b2r = nc.declare_dram_parameter("b2r", [BPC, 1], f32, isOutput=False)
    pos = nc.declare_dram_parameter("pos", [L, 1], f32, isOutput=False)
    lenf = nc.declare_dram_parameter("lenf", [BPC, 1], f32, isOutput=False)
    out = nc.declare_dram_parameter("out", [BPC, L, C], f32, isOutput=True)
    olen = nc.declare_dram_parameter("olen", [BPC, 1], f32, isOutput=True)
    cm_dram = nc.dram_tensor("cm_scratch", [BPC, T], f32)

    with ExitStack() as ctx:
        tc = ctx.enter_context(tile.TileContext(nc))
        singles = ctx.enter_context(tc.tile_pool(name="singles", bufs=1))
        xpool = ctx.enter_context(tc.tile_pool(name="xpool", bufs=5))
        rxpool = ctx.enter_context(tc.tile_pool(name="rxpool", bufs=5))
        xtpool = ctx.enter_context(tc.tile_pool(name="xtpool", bufs=18))
        hpool = ctx.enter_context(tc.tile_pool(name="hpool", bufs=6))
        gpool = ctx.enter_context(tc.tile_pool(name="gpool", bufs=1))
        wpool = ctx.enter_context(tc.tile_pool(name="wpool", bufs=2))
        wtpool = ctx.enter_context(tc.tile_pool(name="wtpool", bufs=18))
        opool = ctx.enter_context(tc.tile_pool(name="opool", bufs=2))
        smalls = ctx.enter_context(tc.tile_pool(name="smalls", bufs=4))
        growp = ctx.enter_context(tc.tile_pool(name="growp", bufs=2))
        psum_h = ctx.enter_context(tc.tile_pool(name="psum_h", bufs=2, space="PSUM"))
        psum_g = ctx.enter_context(tc.tile_pool(name="psum_g", bufs=2, space="PSUM"))
        psum_o = ctx.enter_context(tc.tile_pool(name="psum_o", bufs=2, space="PSUM"))

        # --- replicated weights / constants ---
        w1t_sb = []
        for cc in range(4):
            t_ = singles.tile([128, C], bf16, tag=f"w1t{cc}")
            nc.sync.dma_start(out=t_, in_=w1t[cc * 128 : (cc + 1) * 128, :])
            w1t_sb.append(t_)
        w2_sb = singles.tile([128, 4], bf16, tag="w2")
        nc.sync.dma_start(out=w2_sb, in_=w2[:, :])
        b1_sb = singles.tile([128, 4], f32, tag="b1")
        nc.sync.dma_start(out=b1_sb, in_=b1[:, :])
        b2_sb = singles.tile([BPC, 1], f32, tag="b2")
        nc.sync.dma_start(out=b2_sb, in_=b2r[:, :])
        pos_sb = singles.tile([L, 1], f32, tag="pos")
        nc.sync.dma_start(out=pos_sb, in_=pos[:, :])
        lenf_sb = singles.tile([BPC, 1], f32, tag="lenf")
        nc.sync.dma_start(out=lenf_sb, in_=lenf[:, :])
        iota_sb = singles.tile([BPC, T], f32, tag="iota")
        nc.gpsimd.iota(
            iota_sb,
            pattern=[[1, T]],
            base=0,
            channel_multiplier=0,
            allow_small_or_imprecise_dtypes=True,
        )

        g_sb = gpool.tile([BPC, T], f32, tag="g")  # sigmoid(conv2) per (b, t)

        # --- phase A+B: conv1 + conv2 for every batch element ---
        for b in range(BPC):
            rx = []
            for cc in range(4):
                xb = xpool.tile([128, T], bf16, tag="xbf")
                nc.sync.dma_start(out=xb, in_=xbf[b, cc * 128 : (cc + 1) * 128, :])
                r = rxpool.tile([128, T], bf16, tag="rx")
                nc.vector.tensor_scalar_max(r, xb, 0.0)
                rx.append(r)
            g_row = growp.tile([1, T], f32, tag="grow")
            for tt in range(4):
                ts_ = slice(tt * 512, (tt + 1) * 512)
                pg = psum_g.tile([1, 512], f32, tag="pg")
                for oc in range(4):
                    os_ = slice(oc * 128, (oc + 1) * 128)
                    ph = psum_h.tile([128, 512], f32, tag="ph")
                    for cc in range(4):
                        nc.tensor.matmul(
                            ph,
                            w1t_sb[cc][:, os_],
                            rx[cc][:, ts_],
                            start=(cc == 0),
                            stop=(cc == 3),
                        )
                    h = hpool.tile([128, 512], bf16, tag="h")
                    # h = max(psum + b1, 0), cast to bf16
                    nc.vector.tensor_scalar(
                        h, ph, b1_sb[:, oc : oc + 1], 0.0, Alu.add, Alu.max
                    )
                    nc.tensor.matmul(
                        pg,
                        w2_sb[:, oc : oc + 1],
                        h,
                        start=(oc == 0),
                        stop=(oc == 3),
                    )
                nc.scalar.activation(
                    g_row[:, ts_],
                    pg,
                    Act.Sigmoid,
                    bias=b2_sb[0:1, :],
                    scale=1.0,
                )
            # partition shift (row 0 -> row b) has to go through DMA
            nc.sync.dma_start(out=g_sb[b : b + 1, :], in_=g_row)

        # --- phase C: gate mask, cumsum, centers (all 4 batch rows at once) ---
        gm = gpool.tile([BPC, T], f32, tag="gm")
        # gm = (iota < len) * g
        nc.vector.scalar_tensor_tensor(
            gm, iota_sb, lenf_sb, g_sb, Alu.is_lt, Alu.mult
        )
        ends = gpool.tile([BPC, T], f32, tag="ends")
        nc.vector.tensor_tensor_scan(ends, gm, gm, 0.0, Alu.add, Alu.bypass)
        nc.sync.dma_start(out=olen[:, :], in_=ends[:, T - 1 : T])
        cm = gpool.tile([BPC, T], f32, tag="cm")
        # cm = ends - 0.5*gm
        nc.vector.scalar_tensor_tensor(
            cm, gm, -0.5, ends, Alu.mult, Alu.add
        )
        # gm reused: (iota >= len) * 1e6 ; cm += that  (pushes masked-out
        # positions far from every out position so exp underflows to 0)
        nc.vector.tensor_scalar(gm, iota_sb, lenf_sb, 1e6, Alu.is_ge, Alu.mult)
        nc.vector.tensor_tensor(cm, cm, gm, Alu.add)
        # bounce masked centers through DRAM so they can be partition-
        # broadcast below (SBUF DMA sources need a nonzero partition step)
        nc.sync.dma_start(out=cm_dram[:, :], in_=cm)

        # --- phase D: per-batch softmax attention + output matmul ---
        for b in range(BPC):
            xts = []
            for k in range(16):
                xt_t = xtpool.tile([128, C], bf16, tag="xt")
                nc.sync.dma_start(out=xt_t, in_=xt[b, k * 128 : (k + 1) * 128, :])
                xts.append(xt_t)
            # broadcast centers row b across L partitions (DRAM source)
            cb = wpool.tile([L, T], f32, tag="cb")
            row = cm_dram[b : b + 1, :]
            src = bass.AP(tensor=row.tensor, offset=row.offset, ap=[[0, L]] + list(row.ap[1:]))
            nc.sync.dma_start(out=cb, in_=src)
            nc.vector.tensor_scalar(cb, cb, pos_sb[:, :], None, Alu.subtract)
            d2 = wpool.tile([L, T], f32, tag="d2")
            nc.vector.tensor_tensor(d2, cb, cb, Alu.mult)
            ew = wpool.tile([L, T], bf16, tag="ew")
            sums = smalls.tile([L, 1], f32, tag="sums")
            nc.scalar.activation(
                ew, d2, Act.Exp, bias=0.0, scale=-1.0 / SIGMA2, accum_out=sums
            )
            rsum = smalls.tile([L, 1], f32, tag="rsum")
            nc.vector.reciprocal(rsum, sums)
            po = psum_o.tile([L, C], f32, tag="po")
            for k in range(16):
                wt_t = wtpool.tile([128, L], bf16, tag="wt")
                nc.sync.dma_start(
                    out=wt_t, in_=ew[:, k * 128 : (k + 1) * 128], transpose=True
                )
                nc.tensor.matmul(po, wt_t, xts[k], start=(k == 0), stop=(k == 15))
            ob = opool.tile([L, C], f32, tag="ob")
            nc.vector.tensor_scalar_mul(ob, po, rsum)
            nc.sync.dma_start(out=out[b], in_=ob)

    nc.compile()
    _cache["nc"] = nc
    return nc


def _in_maps(x, len_fea, conv1_w, conv1_b, conv2_w, conv2_b):
    import ml_dtypes

    bf16 = ml_dtypes.bfloat16
    x = np.asarray(x, np.float32)
    x_bf = x.astype(bf16)  # [B, C, T]
    xt_bf = np.ascontiguousarray(x_bf.transpose(0, 2, 1))  # [B, T, C]
    w1t_h = np.ascontiguousarray(np.asarray(conv1_w, np.float32).T).astype(bf16)
    w2_h = np.ascontiguousarray(np.asarray(conv2_w, np.float32).reshape(4, 128).T).astype(bf16)
    b1_h = np.ascontiguousarray(np.asarray(conv1_b, np.float32).reshape(4, 128).T)
    b2_h = np.full((BPC, 1), float(np.asarray(conv2_b)), np.float32)
    pos_h = (0.5 + np.arange(L, dtype=np.float32)).reshape(L, 1)
    lenf_h = np.asarray(len_fea).astype(np.float32).reshape(B, 1)
    maps = []
    for i in range(NCORES):
        s = slice(i * BPC, (i + 1) * BPC)
        maps.append(
            {
                "xbf": np.ascontiguousarray(x_bf[s]),
                "xt": np.ascontiguousarray(xt_bf[s]),
                "w1t": w1t_h,
                "w2": w2_h,
                "b1": b1_h,
                "b2r": b2_h,
                "pos": pos_h,
                "lenf": np.ascontiguousarray(lenf_h[s]),
            }
        )
    return maps


def _install_ntff_shim():
    """Provide antenv.axon_hooks (NTFF profile hook) when the image's
    antenv package lacks it, driving profiling via ctypes into
    libaxon_pjrt.so.  Needed only for BASS_TRACE=1 profiling runs."""
    import types
    import ctypes
    import contextlib

    try:
        from antenv.axon_hooks import get_axon_ntff_profile_hook  # noqa: F401

        return
    except ImportError:
        pass

    holder = {"h": None}
    mod = types.ModuleType("antenv.axon_hooks")
    mod.set_axon_ntff_profile_hook = lambda h: holder.__setitem__("h", h)
    mod.get_axon_ntff_profile_hook = lambda: holder["h"]
    sys.modules["antenv.axon_hooks"] = mod

    so_path = "/opt/axon/libaxon_pjrt.so"
    if not os.path.exists(so_path):
        return
    lib = ctypes.CDLL(so_path)
    if not hasattr(lib, "axon_start_nrt_profile"):
        return
    lib.axon_start_nrt_profile.argtypes = [
        ctypes.POINTER(ctypes.c_int64),
        ctypes.c_size_t,
    ]
    lib.axon_start_nrt_profile.restype = ctypes.c_int64
    lib.axon_stop_nrt_profile.argtypes = [ctypes.c_char_p]
    lib.axon_stop_nrt_profile.restype = ctypes.c_int64

    @contextlib.contextmanager
    def _hook(output_dir, device_ids):
        import jax

        jax.devices()
        if device_ids:
            ids = (ctypes.c_int64 * len(device_ids))(*device_ids)
            rc = lib.axon_start_nrt_profile(ids, len(device_ids))
        else:
            rc = lib.axon_start_nrt_profile(None, 0)
        if rc != 0:
            raise RuntimeError(f"axon_start_nrt_profile rc={rc}")
        try:
            yield
        finally:
            n = lib.axon_stop_nrt_profile(str(output_dir).encode())
            print(f"ntff profile: {n} file(s) written to {output_dir}", file=sys.stderr)

    mod.set_axon_ntff_profile_hook(_hook)


def kernel(x, len_fea, conv1_w, conv1_b, conv2_w, conv2_b):
    global LAST_RESULTS
    *_, bass_utils = _concourse()
    _install_ntff_shim()
    nc = _build()
    maps = _in_maps(x, len_fea, conv1_w, conv1_b, conv2_w, conv2_b)
    res = bass_utils.run_bass_kernel_spmd(nc, maps, core_ids=list(range(NCORES)))
    LAST_RESULTS = res
    feats = np.concatenate([np.asarray(r["out"]) for r in res.results], axis=0)
    lens = np.concatenate([np.asarray(r["olen"])[:, 0] for r in res.results], axis=0)
    return feats.astype(np.float32), lens.astype(np.float32)
